# revision 12
# baseline (speedup 1.0000x reference)
"""Trainium2 Bass kernel for CrossMerge3D.

Input ys: [B=2, S=12, C=96, 32, 32, 32] f32. For each (b, c):
  out = (m0 + perm_j(m1) + perm_k(m2)) / 12
where, with the 12 scans split into 3 groups of 4, each group combines as
  m_g = s0 + s1 + flip(s2 + s3)   (flip over the flattened 32^3 volume)
and group 1's volume is stored as (j,k,i), group 2's as (k,i,j); perm_j /
perm_k bring them back to (i,j,k).

Sharding: 8 cores = batch (2) x channel quarters (4) -> 24 channels/core.
No cross-core communication.

Per-core layout: 4 channels x 32 leading-spatial -> 128 SBUF partitions,
1024-wide free dim. All loads are plain mergeable scan-pair DMAs (1 MiB,
fast HWDGE descriptor path). The flip splits into a free-dim reversal
(folded into the pair-sum's operand APs) and a partition-block reversal
(fp32r matmul against a block-exchange matrix: 1 cycle/row vs fp32's 4).
The fwd pair-sum also routes through the PE array (identity matmul) so
fwd+flip(rev) combine for free in PSUM, with the final 1/12 folded into
both stationary matrices. perm_j / perm_k are DVE 32x32 block transposes
plus free-dim permuted APs. The result is stored bf16 (half the write
traffic; upcast on host), keeping every other step exact fp32.
Loads are spread across both HWDGE rings (SP + ACT); stores alternate
rings per group to balance bytes.
"""

import numpy as np

_B, _S, _C, _D = 2, 12, 96, 32
_NCORE = 8
_CL = _C // 4          # 24 channels per core
_G = _CL // 4          # 6 macro tiles of 4 channels (128 partitions)
_FREE = _D * _D        # 1024

_nc = None


def _build_program():
    from concourse import bacc, tile, mybir

    f32 = mybir.dt.float32
    f32r = mybir.dt.float32r
    bf16 = mybir.dt.bfloat16
    nc = bacc.Bacc(
        "TRN2", target_bir_lowering=False, debug=False, num_devices=_NCORE
    )
    ys = nc.dram_tensor("ys", [_S, _CL, _D, _D, _D], f32, kind="ExternalInput")
    out = nc.dram_tensor("out", [_CL, _D, _D, _D], bf16, kind="ExternalOutput")
    ysa = ys.ap()
    outa = out.ap()

    with tile.TileContext(nc) as tc:
        with (
            tc.tile_pool(name="const", bufs=1) as cst,
            tc.tile_pool(name="io", bufs=2) as iop,
            tc.tile_pool(name="tmp", bufs=2) as tmp,
            tc.tile_pool(name="ps", bufs=1, space="PSUM") as ps,
        ):
            # Stationaries, both scaled by 1/12 so the final scale is free:
            # jblk: 32-block exchange (anti-diagonal per block), i12: identity
            jblk_f = cst.tile([128, 128], f32, tag="jblk_f", name="jblk_f")
            nc.gpsimd.memset(jblk_f[:], 1.0 / 12.0)
            for b in range(4):
                nc.gpsimd.affine_select(
                    out=jblk_f[32 * b:32 * b + 32, :],
                    in_=jblk_f[32 * b:32 * b + 32, :],
                    compare_op=mybir.AluOpType.is_equal, fill=0.0,
                    base=-(32 * b + 31), pattern=[[1, 128]],
                    channel_multiplier=1,
                )
            i12_f = cst.tile([128, 128], f32, tag="i12_f", name="i12_f")
            nc.gpsimd.memset(i12_f[:], 1.0 / 12.0)
            nc.gpsimd.affine_select(
                out=i12_f[:], in_=i12_f[:],
                compare_op=mybir.AluOpType.is_equal, fill=0.0,
                base=0, pattern=[[1, 128]],
                channel_multiplier=-1,
            )
            # fp32r-rounded copies for the PE array
            jblk = cst.tile([128, 128], f32r, tag="jblk", name="jblk")
            nc.vector.tensor_copy(jblk[:], jblk_f[:])
            i12 = cst.tile([128, 128], f32r, tag="i12", name="i12")
            nc.vector.tensor_copy(i12[:], i12_f[:])


            def issue_loads(g):
                cs = slice(4 * g, 4 * (g + 1))

                def load_pair(s, tag, eng):
                    t = iop.tile([128, 2 * _FREE], f32, tag=tag, name=tag,
                                 bufs=2)
                    src = ysa[s:s + 2, cs].rearrange(
                        "s c i j k -> (c i) s (j k)"
                    )
                    dst = t[:].rearrange("p (s f) -> p s f", s=2)
                    eng.dma_start(out=dst, in_=src)
                    return t

                return (
                    load_pair(0, "pa", nc.sync),
                    load_pair(2, "pr", nc.scalar),
                    load_pair(4, "qa", nc.sync),
                    load_pair(6, "qr", nc.scalar),
                    load_pair(8, "ra", nc.sync),
                    load_pair(10, "rr", nc.scalar),
                )

            tiles = {0: issue_loads(0)}
            for g in range(_G):
                cs = slice(4 * g, 4 * (g + 1))
                # prefetch next group's loads BEFORE this group's store in
                # program order, so the store's sem wait on the SP/ACT
                # sequencers never delays load issue
                if g + 1 < _G:
                    tiles[g + 1] = issue_loads(g + 1)
                pa, pr, qa, qr, ra, rr = tiles.pop(g)

                def fwd_sum(t, tag):
                    # f32r out: the add performs the rounding the fp32r
                    # matmul consumer requires
                    fs = tmp.tile([128, _FREE], f32r, tag=tag, name=tag)
                    nc.vector.tensor_add(fs[:], t[:, 0:_FREE],
                                         t[:, _FREE:2 * _FREE])
                    return fs

                def rev_sum(t, tag):
                    # free-dim-reversed pair sum; partition reversal is done
                    # later by the jblk matmul
                    rs = tmp.tile([128, _FREE], f32r, tag=tag, name=tag)
                    nc.vector.tensor_add(rs[:], t[:, 0:_FREE][:, ::-1],
                                         t[:, _FREE:2 * _FREE][:, ::-1])
                    return rs

                fA = fwd_sum(pa, "fA")
                rA = rev_sum(pr, "rA")
                fB = fwd_sum(qa, "fB")
                rB = rev_sum(qr, "rB")
                fC = fwd_sum(ra, "fC")
                rC = rev_sum(rr, "rC")

                def combine(fs, rs, name):
                    # (fwd + partition-block-reversed rev) / 12 in PSUM:
                    # identity matmul accumulates fwd, jblk matmul the rev.
                    # fp32r: 1 cycle/row (exact fp32 MACs).
                    pf = ps.tile([128, _FREE], f32, tag="ps" + name,
                                 name="ps" + name)
                    for n0 in (0, 512):
                        nc.tensor.matmul(pf[:, n0:n0 + 512], i12[:],
                                         fs[:][:, n0:n0 + 512],
                                         start=True, stop=False)
                        nc.tensor.matmul(pf[:, n0:n0 + 512], jblk[:],
                                         rs[:][:, n0:n0 + 512],
                                         start=False, stop=True)
                    return pf

                mA = combine(fA, rA, "A")
                mB = combine(fB, rB, "B")
                mC = combine(fC, rC, "C")

                # group 1 ((j,k,i)): 32x32 block transpose, then add with
                # (k,j)->(j,k) free permute
                tb = tmp.tile([128, _FREE], f32, tag="tb", name="tb")
                nc.vector.transpose(tb[:], mB[:])

                # group 2 ((k,i,j)): (i,j)->(j,i) free permute (ScalarE),
                # then 32x32 block transpose
                cp = tmp.tile([128, _FREE], f32, tag="cp", name="cp")
                rcp = mC[:].rearrange("p (a b) -> p a b", a=_D).transpose(
                    [0, 2, 1]
                )
                nc.scalar.copy(cp[:].rearrange("p (a b) -> p a b", a=_D), rcp)
                tcb = tmp.tile([128, _FREE], f32, tag="tcb", name="tcb")
                nc.vector.transpose(tcb[:], cp[:])

                # final merge: o = mA + perm(tb) + tcb, bf16 out for the store
                t1 = tmp.tile([128, _FREE], f32, tag="t1", name="t1")
                tbp = tb[:].rearrange("p (a b) -> p a b", a=_D).transpose(
                    [0, 2, 1]
                )
                nc.vector.tensor_add(
                    t1[:].rearrange("p (a b) -> p a b", a=_D),
                    mA[:].rearrange("p (a b) -> p a b", a=_D), tbp)
                o = tmp.tile([128, _FREE], bf16, tag="o", name="o")
                nc.vector.tensor_add(o[:], t1[:], tcb[:])

                seng = nc.sync if g % 2 == 0 else nc.scalar
                seng.dma_start(
                    out=outa[cs].rearrange("c i j k -> (c i) (j k)"), in_=o[:]
                )

    nc.compile()
    return nc


def kernel(ys):
    global _nc
    ys = np.ascontiguousarray(ys, dtype=np.float32)
    assert ys.shape == (_B, _S, _C, _D, _D, _D), ys.shape

    if _nc is None:
        _nc = _build_program()

    from concourse.bass_utils import run_bass_kernel_spmd

    in_maps = []
    for r in range(_NCORE):
        b, q = divmod(r, 4)
        shard = np.ascontiguousarray(ys[b, :, q * _CL:(q + 1) * _CL])
        in_maps.append({"ys": shard})

    res = run_bass_kernel_spmd(_nc, in_maps, list(range(_NCORE)))

    out = np.empty((_B, _C, _D, _D, _D), np.float32)
    for r in range(_NCORE):
        b, q = divmod(r, 4)
        out[b, q * _CL:(q + 1) * _CL] = np.asarray(
            res.results[r]["out"]).astype(np.float32)

    if res.exec_time_ns is not None:
        print(f"HW exec time: {res.exec_time_ns} ns")
    return out


# revision 14
# speedup vs baseline: 1.1275x; 1.1275x over previous
"""Trainium2 Bass kernel for CrossMerge3D.

Input ys: [B=2, S=12, C=96, 32, 32, 32] f32. For each (b, c):
  out = (m0 + perm_j(m1) + perm_k(m2)) / 12
where, with the 12 scans split into 3 groups of 4, each group combines as
  m_g = s0 + s1 + flip(s2 + s3)   (flip over the flattened 32^3 volume)
and group 1's volume is stored as (j,k,i), group 2's as (k,i,j); perm_j /
perm_k bring them back to (i,j,k).

Sharding: 8 cores = batch (2) x channel quarters (4) -> 24 channels/core.
No cross-core communication.

Per-core layout: 4 channels x 32 leading-spatial -> 128 SBUF partitions,
1024-wide free dim. All loads are plain mergeable scan-pair DMAs (1 MiB,
fast HWDGE descriptor path). The flip splits into a free-dim reversal
(folded into the pair-sum's operand APs) and a partition-block reversal
(fp32r matmul against a block-exchange matrix: 1 cycle/row vs fp32's 4).
The fwd pair-sum also routes through the PE array (identity matmul) so
fwd+flip(rev) combine for free in PSUM, with the final 1/12 folded into
both stationary matrices. perm_j / perm_k are DVE 32x32 block transposes
plus free-dim permuted APs. The result is stored bf16 (half the write
traffic; upcast on host), keeping every other step exact fp32.
Loads are spread across both HWDGE rings (SP + ACT); stores alternate
rings per group to balance bytes.
"""

import numpy as np

_B, _S, _C, _D = 2, 12, 96, 32
_NCORE = 8
_CL = _C // 4          # 24 channels per core
_G = _CL // 4          # 6 macro tiles of 4 channels (128 partitions)
_FREE = _D * _D        # 1024

_nc = None


def _build_program():
    from concourse import bacc, tile, mybir

    f32 = mybir.dt.float32
    f32r = mybir.dt.float32r
    bf16 = mybir.dt.bfloat16
    nc = bacc.Bacc(
        "TRN2", target_bir_lowering=False, debug=False, num_devices=_NCORE
    )
    ys = nc.dram_tensor("ys", [_S, _CL, _D, _D, _D], f32, kind="ExternalInput")
    out = nc.dram_tensor("out", [_CL, _D, _D, _D], bf16, kind="ExternalOutput")
    ysa = ys.ap()
    outa = out.ap()

    with tile.TileContext(nc) as tc:
        with (
            tc.tile_pool(name="const", bufs=1) as cst,
            tc.tile_pool(name="io", bufs=2) as iop,
            tc.tile_pool(name="tmp", bufs=2) as tmp,
            tc.tile_pool(name="ps", bufs=1, space="PSUM") as ps,
        ):
            # Stationaries, both scaled by 1/12 so the final scale is free:
            # jblk: 32-block exchange (anti-diagonal per block), i12: identity
            jblk_f = cst.tile([128, 128], f32, tag="jblk_f", name="jblk_f")
            nc.gpsimd.memset(jblk_f[:], 1.0 / 12.0)
            for b in range(4):
                nc.gpsimd.affine_select(
                    out=jblk_f[32 * b:32 * b + 32, :],
                    in_=jblk_f[32 * b:32 * b + 32, :],
                    compare_op=mybir.AluOpType.is_equal, fill=0.0,
                    base=-(32 * b + 31), pattern=[[1, 128]],
                    channel_multiplier=1,
                )
            i12_f = cst.tile([128, 128], f32, tag="i12_f", name="i12_f")
            nc.gpsimd.memset(i12_f[:], 1.0 / 12.0)
            nc.gpsimd.affine_select(
                out=i12_f[:], in_=i12_f[:],
                compare_op=mybir.AluOpType.is_equal, fill=0.0,
                base=0, pattern=[[1, 128]],
                channel_multiplier=-1,
            )
            # fp32r-rounded copies for the PE array
            jblk = cst.tile([128, 128], f32r, tag="jblk", name="jblk")
            nc.vector.tensor_copy(jblk[:], jblk_f[:])
            i12 = cst.tile([128, 128], f32r, tag="i12", name="i12")
            nc.vector.tensor_copy(i12[:], i12_f[:])


            def issue_loads(g):
                cs = slice(4 * g, 4 * (g + 1))

                def load_pair(s, tag, eng):
                    t = iop.tile([128, 2 * _FREE], f32, tag=tag, name=tag,
                                 bufs=2)
                    src = ysa[s:s + 2, cs].rearrange(
                        "s c i j k -> (c i) s (j k)"
                    )
                    dst = t[:].rearrange("p (s f) -> p s f", s=2)
                    eng.dma_start(out=dst, in_=src)
                    return t

                # ALL loads on the SP ring: the ACT ring carries only the
                # stores (and ACT's compute), whose long sem waits must not
                # block load issue. Both HWDGE rings feed the same 16 SDMA
                # engines, so bandwidth is unaffected.
                return (
                    load_pair(0, "pa", nc.sync),
                    load_pair(2, "pr", nc.sync),
                    load_pair(4, "qa", nc.sync),
                    load_pair(6, "qr", nc.sync),
                    load_pair(8, "ra", nc.sync),
                    load_pair(10, "rr", nc.sync),
                )

            tiles = {0: issue_loads(0)}
            for g in range(_G):
                cs = slice(4 * g, 4 * (g + 1))
                # prefetch next group's loads BEFORE this group's store in
                # program order, so the store's sem wait on the SP/ACT
                # sequencers never delays load issue
                if g + 1 < _G:
                    tiles[g + 1] = issue_loads(g + 1)
                pa, pr, qa, qr, ra, rr = tiles.pop(g)

                def fwd_sum(t, tag):
                    # f32r out: the add performs the rounding the fp32r
                    # matmul consumer requires
                    fs = tmp.tile([128, _FREE], f32r, tag=tag, name=tag)
                    nc.vector.tensor_add(fs[:], t[:, 0:_FREE],
                                         t[:, _FREE:2 * _FREE])
                    return fs

                def rev_sum(t, tag):
                    # free-dim-reversed pair sum; partition reversal is done
                    # later by the jblk matmul
                    rs = tmp.tile([128, _FREE], f32r, tag=tag, name=tag)
                    nc.vector.tensor_add(rs[:], t[:, 0:_FREE][:, ::-1],
                                         t[:, _FREE:2 * _FREE][:, ::-1])
                    return rs

                fA = fwd_sum(pa, "fA")
                rA = rev_sum(pr, "rA")
                fB = fwd_sum(qa, "fB")
                rB = rev_sum(qr, "rB")
                fC = fwd_sum(ra, "fC")
                rC = rev_sum(rr, "rC")

                def combine(fs, rs, name):
                    # (fwd + partition-block-reversed rev) / 12 in PSUM:
                    # identity matmul accumulates fwd, jblk matmul the rev.
                    # fp32r: 1 cycle/row (exact fp32 MACs).
                    pf = ps.tile([128, _FREE], f32, tag="ps" + name,
                                 name="ps" + name)
                    for n0 in (0, 512):
                        nc.tensor.matmul(pf[:, n0:n0 + 512], i12[:],
                                         fs[:][:, n0:n0 + 512],
                                         start=True, stop=False)
                        nc.tensor.matmul(pf[:, n0:n0 + 512], jblk[:],
                                         rs[:][:, n0:n0 + 512],
                                         start=False, stop=True)
                    return pf

                mA = combine(fA, rA, "A")
                mB = combine(fB, rB, "B")
                mC = combine(fC, rC, "C")

                # group 1 ((j,k,i)): 32x32 block transpose, then add with
                # (k,j)->(j,k) free permute
                tb = tmp.tile([128, _FREE], f32, tag="tb", name="tb")
                nc.vector.transpose(tb[:], mB[:])

                # group 2 ((k,i,j)): (i,j)->(j,i) free permute (ScalarE),
                # then 32x32 block transpose
                cp = tmp.tile([128, _FREE], f32, tag="cp", name="cp")
                rcp = mC[:].rearrange("p (a b) -> p a b", a=_D).transpose(
                    [0, 2, 1]
                )
                nc.scalar.copy(cp[:].rearrange("p (a b) -> p a b", a=_D), rcp)
                tcb = tmp.tile([128, _FREE], f32, tag="tcb", name="tcb")
                nc.vector.transpose(tcb[:], cp[:])

                # final merge: o = mA + perm(tb) + tcb, bf16 out for the store
                t1 = tmp.tile([128, _FREE], f32, tag="t1", name="t1")
                tbp = tb[:].rearrange("p (a b) -> p a b", a=_D).transpose(
                    [0, 2, 1]
                )
                nc.vector.tensor_add(
                    t1[:].rearrange("p (a b) -> p a b", a=_D),
                    mA[:].rearrange("p (a b) -> p a b", a=_D), tbp)
                o = tmp.tile([128, _FREE], bf16, tag="o", name="o")
                nc.vector.tensor_add(o[:], t1[:], tcb[:])

                nc.scalar.dma_start(
                    out=outa[cs].rearrange("c i j k -> (c i) (j k)"), in_=o[:]
                )

    nc.compile()
    return nc


def kernel(ys):
    global _nc
    ys = np.ascontiguousarray(ys, dtype=np.float32)
    assert ys.shape == (_B, _S, _C, _D, _D, _D), ys.shape

    if _nc is None:
        _nc = _build_program()

    from concourse.bass_utils import run_bass_kernel_spmd

    in_maps = []
    for r in range(_NCORE):
        b, q = divmod(r, 4)
        shard = np.ascontiguousarray(ys[b, :, q * _CL:(q + 1) * _CL])
        in_maps.append({"ys": shard})

    res = run_bass_kernel_spmd(_nc, in_maps, list(range(_NCORE)))

    out = np.empty((_B, _C, _D, _D, _D), np.float32)
    for r in range(_NCORE):
        b, q = divmod(r, 4)
        out[b, q * _CL:(q + 1) * _CL] = np.asarray(
            res.results[r]["out"]).astype(np.float32)

    if res.exec_time_ns is not None:
        print(f"HW exec time: {res.exec_time_ns} ns")
    return out


# revision 18
# speedup vs baseline: 1.1288x; 1.0011x over previous
"""Trainium2 Bass kernel for CrossMerge3D.

Input ys: [B=2, S=12, C=96, 32, 32, 32] f32. For each (b, c):
  out = (m0 + perm_j(m1) + perm_k(m2)) / 12
where, with the 12 scans split into 3 groups of 4, each group combines as
  m_g = s0 + s1 + flip(s2 + s3)   (flip over the flattened 32^3 volume)
and group 1's volume is stored as (j,k,i), group 2's as (k,i,j); perm_j /
perm_k bring them back to (i,j,k).

Sharding: 8 cores = batch (2) x channel quarters (4) -> 24 channels/core.
No cross-core communication.

Per-core layout: 4 channels x 32 leading-spatial -> 128 SBUF partitions,
1024-wide free dim. All loads are plain mergeable scan-pair DMAs (1 MiB,
fast HWDGE descriptor path). The flip splits into a free-dim reversal
(folded into the pair-sum's operand APs) and a partition-block reversal
(fp32r matmul against a block-exchange matrix: 1 cycle/row vs fp32's 4).
The fwd pair-sum also routes through the PE array (identity matmul) so
fwd+flip(rev) combine for free in PSUM, with the final 1/12 folded into
both stationary matrices. perm_j / perm_k are DVE 32x32 block transposes
plus free-dim permuted APs. The result is stored bf16 (half the write
traffic; upcast on host), keeping every other step exact fp32.
Loads are spread across both HWDGE rings (SP + ACT); stores alternate
rings per group to balance bytes.
"""

import numpy as np

_B, _S, _C, _D = 2, 12, 96, 32
_NCORE = 8
_CL = _C // 4          # 24 channels per core
_G = _CL // 4          # 6 macro tiles of 4 channels (128 partitions)
_FREE = _D * _D        # 1024

_nc = None


def _build_program():
    from concourse import bacc, tile, mybir

    f32 = mybir.dt.float32
    f32r = mybir.dt.float32r
    bf16 = mybir.dt.bfloat16
    nc = bacc.Bacc(
        "TRN2", target_bir_lowering=False, debug=False, num_devices=_NCORE
    )
    ys = nc.dram_tensor("ys", [_S, _CL, _D, _D, _D], f32, kind="ExternalInput")
    out = nc.dram_tensor("out", [_CL, _D, _D, _D], bf16, kind="ExternalOutput")
    ysa = ys.ap()
    outa = out.ap()

    with tile.TileContext(nc) as tc:
        with (
            tc.tile_pool(name="const", bufs=1) as cst,
            tc.tile_pool(name="io", bufs=2) as iop,
            tc.tile_pool(name="tmp", bufs=2) as tmp,
            tc.tile_pool(name="ps", bufs=1, space="PSUM") as ps,
        ):
            # Stationaries, both scaled by 1/12 so the final scale is free:
            # jblk: 32-block exchange (anti-diagonal per block), i12: identity
            jblk_f = cst.tile([128, 128], f32, tag="jblk_f", name="jblk_f")
            nc.gpsimd.memset(jblk_f[:], 1.0 / 12.0)
            for b in range(4):
                nc.gpsimd.affine_select(
                    out=jblk_f[32 * b:32 * b + 32, :],
                    in_=jblk_f[32 * b:32 * b + 32, :],
                    compare_op=mybir.AluOpType.is_equal, fill=0.0,
                    base=-(32 * b + 31), pattern=[[1, 128]],
                    channel_multiplier=1,
                )
            i12_f = cst.tile([128, 128], f32, tag="i12_f", name="i12_f")
            nc.gpsimd.memset(i12_f[:], 1.0 / 12.0)
            nc.gpsimd.affine_select(
                out=i12_f[:], in_=i12_f[:],
                compare_op=mybir.AluOpType.is_equal, fill=0.0,
                base=0, pattern=[[1, 128]],
                channel_multiplier=-1,
            )
            # fp32r-rounded copies for the PE array
            jblk = cst.tile([128, 128], f32r, tag="jblk", name="jblk")
            nc.vector.tensor_copy(jblk[:], jblk_f[:])
            i12 = cst.tile([128, 128], f32r, tag="i12", name="i12")
            nc.vector.tensor_copy(i12[:], i12_f[:])


            def issue_loads(g):
                cs = slice(4 * g, 4 * (g + 1))

                def load_pair(s, tag, eng):
                    t = iop.tile([128, 2 * _FREE], f32, tag=tag, name=tag,
                                 bufs=2)
                    src = ysa[s:s + 2, cs].rearrange(
                        "s c i j k -> (c i) s (j k)"
                    )
                    dst = t[:].rearrange("p (s f) -> p s f", s=2)
                    eng.dma_start(out=dst, in_=src)
                    return t

                # ALL loads on the SP ring: the ACT ring carries only the
                # stores (and ACT's compute), whose long sem waits must not
                # block load issue. Both HWDGE rings feed the same 16 SDMA
                # engines, so bandwidth is unaffected.
                # Volume C loads first (its merge chain cp->tcb is longest),
                # volume A last (shortest chain) -- shrinks the final tail.
                rr = load_pair(10, "rr", nc.sync)
                ra = load_pair(8, "ra", nc.sync)
                qr = load_pair(6, "qr", nc.sync)
                qa = load_pair(4, "qa", nc.sync)
                pr = load_pair(2, "pr", nc.sync)
                pa = load_pair(0, "pa", nc.sync)
                return (pa, pr, qa, qr, ra, rr)

            tiles = {0: issue_loads(0)}
            for g in range(_G):
                cs = slice(4 * g, 4 * (g + 1))
                # prefetch next group's loads BEFORE this group's store in
                # program order, so the store's sem wait on the SP/ACT
                # sequencers never delays load issue
                if g + 1 < _G:
                    tiles[g + 1] = issue_loads(g + 1)
                pa, pr, qa, qr, ra, rr = tiles.pop(g)

                def fwd_sum(t, tag):
                    # f32r out: the add performs the rounding the fp32r
                    # matmul consumer requires
                    fs = tmp.tile([128, _FREE], f32r, tag=tag, name=tag)
                    nc.vector.tensor_add(fs[:], t[:, 0:_FREE],
                                         t[:, _FREE:2 * _FREE])
                    return fs

                def rev_sum(t, tag):
                    # free-dim-reversed pair sum; partition reversal is done
                    # later by the jblk matmul
                    rs = tmp.tile([128, _FREE], f32r, tag=tag, name=tag)
                    nc.vector.tensor_add(rs[:], t[:, 0:_FREE][:, ::-1],
                                         t[:, _FREE:2 * _FREE][:, ::-1])
                    return rs

                rC = rev_sum(rr, "rC")
                fC = fwd_sum(ra, "fC")
                rB = rev_sum(qr, "rB")
                fB = fwd_sum(qa, "fB")
                rA = rev_sum(pr, "rA")
                fA = fwd_sum(pa, "fA")

                def combine(fs, rs, name):
                    # (fwd + partition-block-reversed rev) / 12 in PSUM:
                    # identity matmul accumulates fwd, jblk matmul the rev.
                    # fp32r: 1 cycle/row (exact fp32 MACs).
                    pf = ps.tile([128, _FREE], f32, tag="ps" + name,
                                 name="ps" + name)
                    for n0 in (0, 512):
                        nc.tensor.matmul(pf[:, n0:n0 + 512], i12[:],
                                         fs[:][:, n0:n0 + 512],
                                         start=True, stop=False)
                        nc.tensor.matmul(pf[:, n0:n0 + 512], jblk[:],
                                         rs[:][:, n0:n0 + 512],
                                         start=False, stop=True)
                    return pf

                mC = combine(fC, rC, "C")
                mB = combine(fB, rB, "B")
                mA = combine(fA, rA, "A")

                # group 2 ((k,i,j)): (i,j)->(j,i) free permute (ScalarE),
                # then 32x32 block transpose -- issued first, C loads first
                cp = tmp.tile([128, _FREE], f32, tag="cp", name="cp")
                rcp = mC[:].rearrange("p (a b) -> p a b", a=_D).transpose(
                    [0, 2, 1]
                )
                nc.scalar.copy(cp[:].rearrange("p (a b) -> p a b", a=_D), rcp)
                tcb = tmp.tile([128, _FREE], f32, tag="tcb", name="tcb")
                nc.vector.transpose(tcb[:], cp[:])

                # group 1 ((j,k,i)): 32x32 block transpose, then add with
                # (k,j)->(j,k) free permute
                tb = tmp.tile([128, _FREE], f32, tag="tb", name="tb")
                nc.vector.transpose(tb[:], mB[:])

                # final merge: o = mA + perm(tb) + tcb, bf16 out for the store
                t1 = tmp.tile([128, _FREE], f32, tag="t1", name="t1")
                tbp = tb[:].rearrange("p (a b) -> p a b", a=_D).transpose(
                    [0, 2, 1]
                )
                nc.vector.tensor_add(
                    t1[:].rearrange("p (a b) -> p a b", a=_D),
                    mA[:].rearrange("p (a b) -> p a b", a=_D), tbp)
                o = tmp.tile([128, _FREE], bf16, tag="o", name="o")
                nc.vector.tensor_add(o[:], t1[:], tcb[:])

                nc.scalar.dma_start(
                    out=outa[cs].rearrange("c i j k -> (c i) (j k)"), in_=o[:]
                )

    nc.compile()
    return nc


def kernel(ys):
    global _nc
    ys = np.ascontiguousarray(ys, dtype=np.float32)
    assert ys.shape == (_B, _S, _C, _D, _D, _D), ys.shape

    if _nc is None:
        _nc = _build_program()

    from concourse.bass_utils import run_bass_kernel_spmd

    in_maps = []
    for r in range(_NCORE):
        b, q = divmod(r, 4)
        shard = np.ascontiguousarray(ys[b, :, q * _CL:(q + 1) * _CL])
        in_maps.append({"ys": shard})

    res = run_bass_kernel_spmd(_nc, in_maps, list(range(_NCORE)))

    out = np.empty((_B, _C, _D, _D, _D), np.float32)
    for r in range(_NCORE):
        b, q = divmod(r, 4)
        out[b, q * _CL:(q + 1) * _CL] = np.asarray(
            res.results[r]["out"]).astype(np.float32)

    if res.exec_time_ns is not None:
        print(f"HW exec time: {res.exec_time_ns} ns")
    return out
